# revision 29
# baseline (speedup 1.0000x reference)
"""Trainium2 Bass kernel for nn_HFGA_54606214201918.

Computation (per batch element b, C=256 channels, L=4096 positions):
    xh  = (x[:, 0::2] - x[:, 1::2]) / sqrt(2)          # Haar high band  [C, L/2]
    q   = Wq @ x + bq                                  # [C, L]
    k   = Wk @ xh + bk                                 # [C, L/2]
    v   = Wv @ xh + bv                                 # [C, L/2]
    attn = softmax_over_keys((k^T q) / sqrt(C))        # [L/2, L]
    out = (v @ attn) * tanh(gate) + x

Sharding: data-parallel over batch B=8 across the 8 NeuronCores (one batch
element per core); weights are broadcast. No collectives needed.

Per-core algorithm:
  - The two big matmuls (scores k^T q and yh = v @ E) plus the K/V projections
    run in fp8e4m3 with DoubleRow perf mode (K=256 contraction per MM, 2
    MACs/cell/cycle). The Q projection runs in float32r directly from the
    fp32 x tile (saves a DVE quantize pass over x).
  - scores land in [keys m, queries l] layout, two 128-key chunks per 2-bank
    PSUM tile, so one ACTIVATE(Exp) drains 1024 f32/partition per instruction
    and emits the fp8 E pair tile in exactly the DoubleRow rhs layout.
  - 1/sqrt(C) and the softmax shift (-3, range-fitting exp into fp8) ride the
    ACT free affine: E = exp(s/16 - 3); the shift cancels in softmax.
  - softmax denominator Z[l] via a DoubleRow ones-column matmul accumulated
    over key pairs; normalization applied to the small output (v@E) with a
    K=1 f32r broadcast matmul, fused with the residual add on the DVE.
  - Prologue pipelines per 512-column x bank: DMA -> Haar sub -> K/V
    projections, so the PE starts ~2.5us in. Q projection for l-tile lt+1 is
    emitted inside l-tile lt's attention loop.
  - 1/sqrt(2) and tanh(gate) are folded into Wk/Wv/bv on host.
"""
import sys

if '/opt/trn_rl_repo' not in sys.path:
    sys.path.insert(0, '/opt/trn_rl_repo')

import numpy as np

import concourse.bass as bass
import concourse.tile as tile
from concourse import bacc, mybir
from concourse import bass_utils

B, C, L = 8, 256, 4096
M = L // 2            # 2048 keys
P = 128               # partitions
CO = C // P           # 2 channel chunks
LB = 512              # l-tile (one PSUM bank of fp32)
NB = L // LB          # 8 l-tiles
MJ = M // P           # 16 key chunks
NPAIR = MJ // 2       # 8 key pair-chunks
INV_SQRT2 = 0.7071067811865476

F32 = mybir.dt.float32
F16 = mybir.dt.float16
F32R = mybir.dt.float32r
BF16 = mybir.dt.bfloat16
F8 = mybir.dt.float8e4
AF = mybir.ActivationFunctionType
DR = mybir.MatmulPerfMode.DoubleRow

EXP_SHIFT = -3.0      # softmax-invariant shift to fit E into fp8e4m3
EXP_SCALE = 1.0 / 16.0  # 1/sqrt(C)

_CACHE = {}


def _build():
    nc = bacc.Bacc("TRN2", target_bir_lowering=False, debug=False, num_devices=8)

    x_d = nc.dram_tensor("x", [C, L], F16, kind="ExternalInput").ap()
    wq_d = nc.dram_tensor("wqT", [C, C], F8, kind="ExternalInput").ap()
    wk_d = nc.dram_tensor("wkT", [C, C], F8, kind="ExternalInput").ap()
    wv_d = nc.dram_tensor("wvT", [C, C], F8, kind="ExternalInput").ap()
    bq_d = nc.dram_tensor("bq", [C], F32, kind="ExternalInput").ap()
    bk_d = nc.dram_tensor("bk", [C], F32, kind="ExternalInput").ap()
    bv_d = nc.dram_tensor("bvt", [C], BF16, kind="ExternalInput").ap()
    y_d = nc.dram_tensor("y", [C, L], F32, kind="ExternalOutput").ap()

    x3 = x_d.rearrange("(co ci) l -> ci co l", ci=P)      # [128, 2, 4096]
    y3 = y_d.rearrange("(co ci) l -> ci co l", ci=P)
    wq3 = wq_d.rearrange("(cc ci) o -> ci cc o", ci=P)    # [128, 2, 256] lhsT
    wk3 = wk_d.rearrange("(cc ci) o -> ci cc o", ci=P)
    wv3 = wv_d.rearrange("(cc ci) o -> ci cc o", ci=P)
    bq2 = bq_d.rearrange("(oc oi) -> oi oc", oi=P)        # [128, 2]
    bk2 = bk_d.rearrange("(oc oi) -> oi oc", oi=P)

    with tile.TileContext(nc) as tc:
        with tc.tile_pool(name="consts", bufs=1) as consts, \
             tc.tile_pool(name="big", bufs=1) as big, \
             tc.tile_pool(name="e", bufs=4) as e_pool, \
             tc.tile_pool(name="tmp", bufs=3) as tmp_pool, \
             tc.tile_pool(name="outp", bufs=2) as out_pool, \
             tc.tile_pool(name="ps", bufs=1, space="PSUM") as ps:

            # ---- constants (host pre-quantized to fp8/bf16) ----
            bq_sb = consts.tile([P, CO], F32)
            bk_sb = consts.tile([P, CO], F32)
            wq_r = consts.tile([P, CO, C], F8)            # DR lhsT/rhs
            wk_r = consts.tile([P, CO, C], F8)
            wv_r = consts.tile([P, CO, C], F8)
            bv_bf = consts.tile([1, C], BF16)

            ones_st = consts.tile([P, CO, 16], F32)
            nc.vector.memset(ones_st, 1.0)
            ones_dr = consts.tile([P, CO, 16], F8)        # DR ones lhsT for Z
            nc.vector.tensor_copy(ones_dr, ones_st)
            ones_row_f = consts.tile([1, P], F32)         # K=1 bcast lhsT (f32r)
            nc.vector.memset(ones_row_f, 1.0)
            ones_row_bf = consts.tile([1, P], BF16)       # bias rank-1 lhsT
            nc.vector.tensor_copy(ones_row_bf, ones_row_f)

            shift_col = consts.tile([P, 1], F32)          # exp bias column
            nc.vector.memset(shift_col, EXP_SHIFT)

            # preload the exp table set during the prologue
            dm = tmp_pool.tile([1, 16], F32, tag="rz", name="dummy_exp")
            nc.scalar.activation(dm, ones_st[0:1, 0, :], AF.Exp,
                                 bias=shift_col[0:1, :])

            # ---- big persistent tensors ----
            x_sb = big.tile([P, NB, CO, LB], F16)         # bank-major fp16
            q_sb = big.tile([P, NB, CO, LB], F8)          # [o, l] bank-major
            xh_sb = big.tile([P, NB, CO, LB // 2], F8)    # [c, m] bank-major
            k_sb = big.tile([P, NB, CO, LB // 2], F8)     # [o, m] bank-major
            vt_sb = big.tile([P, MJ, C], F8)              # [m, o] chunks

            xr_tiles = {}
            vp_tiles = {}

            def xr_cast(lt):
                xr = e_pool.tile([P, CO, LB], F8, tag="xr", bufs=3,
                                 name=f"xr{lt}")
                nc.vector.tensor_copy(xr, x_bank(lt))
                xr_tiles[lt] = xr

            def q_proj(lt):
                sl = slice(lt * LB, (lt + 1) * LB)
                xr = xr_tiles.pop(lt)
                for oc in range(CO):
                    qp = ps.tile([P, LB], F32, tag="qb", bufs=1,
                                 name=f"qp{lt}_{oc}")
                    nc.tensor.matmul(qp, wq_r[:, :, oc * P:(oc + 1) * P],
                                     xr,
                                     start=True, stop=True, perf_mode=DR)
                    nc.vector.tensor_scalar_add(
                        q_sb[:, lt, oc, :], qp, bq_sb[:, oc:oc + 1])

            # ---- prologue DMAs: tiny fp8 weights land first; x half-bank
            # transfers round-robin over the 3 DMA-capable queues ----
            nc.sync.dma_start(out=wk_r, in_=wk3)
            nc.gpsimd.dma_start(out=wv_r, in_=wv3)
            nc.scalar.dma_start(out=wq_r, in_=wq3)
            nc.scalar.dma_start(out=bq_sb, in_=bq2)
            nc.scalar.dma_start(out=bk_sb, in_=bk2)
            nc.scalar.dma_start(out=bv_bf, in_=bv_d[None, :])
            # x halves: sync takes co0 of banks 0-6, gpsimd co1 of 0-6 plus
            # bank7 co0; scalar only 2 late halves (>6 DMAs on a queue makes
            # dma_start block on semaphore recycling, and scalar must be free
            # early for the vt copies / exps)
            xq = {}
            for j in range(7):
                xq[(j, 0)] = nc.sync
                xq[(j, 1)] = nc.gpsimd
            xq[(7, 0)] = nc.gpsimd
            xq[(6, 1)] = nc.sync
            xq[(6, 0)] = nc.scalar
            xq[(7, 1)] = nc.scalar
            for idx in range(2 * NB):
                j, co = idx // 2, idx % 2
                sl = slice(j * LB, (j + 1) * LB)
                xq[(j, co)].dma_start(out=x_sb[:, j, co, :],
                                      in_=x3[:, co, sl])

            def x_bank(j):
                return x_sb[:, j]

            # ---- per-bank prologue: Haar; K/V proj (fused into lt0) ----
            def bank_prologue(j):
                sl = slice(j * LB, (j + 1) * LB)
                msl = slice(j * (LB // 2), (j + 1) * (LB // 2))  # 256 keys
                # Haar high band (no 1/sqrt2: folded into wk/wv)
                pair = x_bank(j).rearrange(
                    "p c (m two) -> p c m two", two=2)
                nc.vector.tensor_sub(xh_sb[:, j], pair[:, :, :, 0],
                                     pair[:, :, :, 1])

                # K projection: k[o, m] += wk^T xh  (DR over channels)
                kp = ps.tile([P, 2 * LB], F32, tag="score", bufs=2,
                             name=f"kp{j}")
                for oc in range(CO):
                    nc.tensor.matmul(
                        kp[:, oc * 256:(oc + 1) * 256],
                        wk_r[:, :, oc * P:(oc + 1) * P],
                        xh_sb[:, j],
                        start=True, stop=True, perf_mode=DR)
                for oc in range(CO):
                    nc.vector.tensor_scalar_add(
                        k_sb[:, j, oc, :], kp[:, oc * 256:(oc + 1) * 256],
                        bk_sb[:, oc:oc + 1])

                if j == 0:
                    xr_cast(0)
                    q_proj(0)

                # V^T projection: vt[m, o] = xh^T wv + bv (DR over channels)
                vp = ps.tile([P, 512], F32, tag="qb", bufs=1, name=f"vp{j}")
                for t in range(2):
                    mj = 2 * j + t
                    nc.tensor.matmul(
                        vp[:, t * 256:(t + 1) * 256],
                        xh_sb[:, j, :, t * P:(t + 1) * P],
                        wv_r,
                        start=True, stop=False, perf_mode=DR)
                    nc.tensor.matmul(
                        vp[:, t * 256:(t + 1) * 256],
                        ones_row_bf, bv_bf, start=False, stop=True)
                vp_tiles[j] = vp

            # ---- attention, one l-tile (512 queries) at a time ----
            LAG = 1
            for lt in range(NB):
                sl = slice(lt * LB, (lt + 1) * LB)
                zp = ps.tile([1, LB], F32, tag="z", bufs=1, name=f"zp{lt}")
                yhp = [ps.tile([P, LB], F32, tag="yh", bufs=2,
                               name=f"yh{lt}_{i}") for i in range(CO)]
                pend = {}

                def consume(t):
                    e2v = pend.pop(t).rearrange("p (two l) -> p two l", two=2)
                    nc.tensor.matmul(zp, ones_dr[:, :, 0:1], e2v,
                                     start=(t == 0), stop=(t == NPAIR - 1),
                                     perf_mode=DR)
                    for oc in range(CO):
                        nc.tensor.matmul(
                            yhp[oc], vt_sb[:, 2 * t:2 * t + 2,
                                           oc * P:(oc + 1) * P], e2v,
                            start=(t == 0), stop=(t == NPAIR - 1),
                            perf_mode=DR)

                for t in range(NPAIR):
                    if lt == 0:
                        # gate lt0's per-bank work at the real (not modeled)
                        # DMA arrival time of bank t so the static schedule
                        # doesn't head-block on late banks
                        arr = 0.008 + 0.0023 * t
                        with tc.tile_wait_until(arr):
                            bank_prologue(t)
                            if t == 1:
                                xr_cast(1)
                    sp = ps.tile([P, 2 * LB], F32, tag="score", bufs=2,
                                 name=f"sp{lt}_{t}")
                    for h in range(2):
                        nc.tensor.matmul(
                            sp[:, h * LB:(h + 1) * LB],
                            k_sb[:, t, :, h * P:(h + 1) * P],
                            q_sb[:, lt],
                            start=True, stop=True, perf_mode=DR)
                    if t == 3 and lt + 2 < NB:
                        xr_cast(lt + 2)
                    e2 = e_pool.tile([P, 2 * LB], F8, tag="e",
                                     name=f"e{lt}_{t}")
                    nc.scalar.activation(e2, sp, AF.Exp,
                                         bias=shift_col, scale=EXP_SCALE)
                    pend[t] = e2
                    if lt == 0:
                        vp = vp_tiles.pop(t)
                        for h in range(2):
                            nc.scalar.copy(vt_sb[:, 2 * t + h, :],
                                           vp[:, h * 256:(h + 1) * 256])
                    if t == 1 and lt + 1 < NB:
                        q_proj(lt + 1)
                    if t >= LAG:
                        consume(t - LAG)
                for t in range(NPAIR - LAG, NPAIR):
                    consume(t)

                # normalize + residual
                bpp = ps.tile([P, LB], F32, tag="qb", bufs=1, name=f"bp{lt}")
                rz = tmp_pool.tile([1, LB], F32, tag="rz", name=f"rz{lt}")
                nc.vector.reciprocal_approx_fast(out=rz, in_=zp)
                rz_bf = tmp_pool.tile([1, LB], BF16, tag="rzb",
                                      name=f"rzb{lt}")
                nc.vector.tensor_copy(rz_bf, rz)
                nc.tensor.matmul(bpp, ones_row_bf, rz_bf,
                                 start=True, stop=True)
                b_sb = tmp_pool.tile([P, LB], F32, tag="bsb", name=f"bsb{lt}")
                nc.vector.tensor_copy(b_sb, bpp)
                o_sb = out_pool.tile([P, CO, LB], F32, tag="o",
                                     name=f"o{lt}")
                for oc in range(CO):
                    t_sb = tmp_pool.tile([P, LB], F32, tag="t",
                                         name=f"t{lt}_{oc}")
                    nc.vector.tensor_mul(t_sb, yhp[oc], b_sb)
                    nc.vector.tensor_add(o_sb[:, oc, :], t_sb,
                                         x_bank(lt)[:, oc, :])
                    (nc.sync, nc.gpsimd)[(lt + oc) % 2].dma_start(
                        out=y3[:, oc, sl], in_=o_sb[:, oc, :])

    nc.compile()
    return nc


def _get_nc():
    if "nc" not in _CACHE:
        _CACHE["nc"] = _build()
    return _CACHE["nc"]


def _prep_inputs(x, Wq, bq, Wk, bk, Wv, bv, attn_gate):
    x = np.asarray(x, dtype=np.float32)
    Wq = np.asarray(Wq, dtype=np.float32)
    Wk = np.asarray(Wk, dtype=np.float32)
    Wv = np.asarray(Wv, dtype=np.float32)
    bq = np.asarray(bq, dtype=np.float32)
    bk = np.asarray(bk, dtype=np.float32)
    bv = np.asarray(bv, dtype=np.float32)
    gate = float(np.tanh(np.asarray(attn_gate, dtype=np.float64))[0])

    import ml_dtypes
    f8np = ml_dtypes.float8_e4m3
    # lhsT layouts [c_in, c_out]; fold haar 1/sqrt(2) into k and v,
    # tanh(gate) into v.  1/sqrt(C) rides the exp free-affine on device.
    # Weights pre-quantized on host to the device matmul dtypes.
    wqT = np.ascontiguousarray(Wq.T).astype(f8np)
    wkT = np.ascontiguousarray(Wk.T * np.float32(INV_SQRT2)).astype(f8np)
    wvT = np.ascontiguousarray(Wv.T * np.float32(INV_SQRT2 * gate)).astype(
        f8np)
    bv_t = (bv * np.float32(gate)).astype(ml_dtypes.bfloat16)
    return [{
        "x": np.ascontiguousarray(x[b]).astype(np.float16),
        "wqT": wqT, "wkT": wkT, "wvT": wvT,
        "bq": bq, "bk": bk, "bvt": bv_t,
    } for b in range(B)]


def kernel(x, Wq, bq, Wk, bk, Wv, bv, attn_gate, _run_kwargs=None):
    in_maps = _prep_inputs(x, Wq, bq, Wk, bk, Wv, bv, attn_gate)
    nc = _get_nc()
    res = bass_utils.run_bass_kernel_spmd(
        nc, in_maps, core_ids=list(range(B)), **(_run_kwargs or {}))
    out = np.stack([res.results[b]["y"] for b in range(B)]).astype(np.float32)
    if _run_kwargs:
        kernel.last_results = res
    return out


# revision 31
# speedup vs baseline: 1.0676x; 1.0676x over previous
"""Trainium2 Bass kernel for nn_HFGA_54606214201918.

Computation (per batch element b, C=256 channels, L=4096 positions):
    xh  = (x[:, 0::2] - x[:, 1::2]) / sqrt(2)          # Haar high band  [C, L/2]
    q   = Wq @ x + bq                                  # [C, L]
    k   = Wk @ xh + bk                                 # [C, L/2]
    v   = Wv @ xh + bv                                 # [C, L/2]
    attn = softmax_over_keys((k^T q) / sqrt(C))        # [L/2, L]
    out = (v @ attn) * tanh(gate) + x

Sharding: data-parallel over batch B=8 across the 8 NeuronCores (one batch
element per core); weights are broadcast. No collectives needed.

Per-core algorithm:
  - The two big matmuls (scores k^T q and yh = v @ E) plus the K/V projections
    run in fp8e4m3 with DoubleRow perf mode (K=256 contraction per MM, 2
    MACs/cell/cycle). The Q projection runs in float32r directly from the
    fp32 x tile (saves a DVE quantize pass over x).
  - scores land in [keys m, queries l] layout, two 128-key chunks per 2-bank
    PSUM tile, so one ACTIVATE(Exp) drains 1024 f32/partition per instruction
    and emits the fp8 E pair tile in exactly the DoubleRow rhs layout.
  - 1/sqrt(C) and the softmax shift (-3, range-fitting exp into fp8) ride the
    ACT free affine: E = exp(s/16 - 3); the shift cancels in softmax.
  - softmax denominator Z[l] via a DoubleRow ones-column matmul accumulated
    over key pairs; normalization applied to the small output (v@E) with a
    K=1 f32r broadcast matmul, fused with the residual add on the DVE.
  - Prologue pipelines per 512-column x bank: DMA -> Haar sub -> K/V
    projections, so the PE starts ~2.5us in. Q projection for l-tile lt+1 is
    emitted inside l-tile lt's attention loop.
  - 1/sqrt(2) and tanh(gate) are folded into Wk/Wv/bv on host.
"""
import sys

if '/opt/trn_rl_repo' not in sys.path:
    sys.path.insert(0, '/opt/trn_rl_repo')

import numpy as np

import concourse.bass as bass
import concourse.tile as tile
from concourse import bacc, mybir
from concourse import bass_utils

B, C, L = 8, 256, 4096
M = L // 2            # 2048 keys
P = 128               # partitions
CO = C // P           # 2 channel chunks
LB = 512              # l-tile (one PSUM bank of fp32)
NB = L // LB          # 8 l-tiles
MJ = M // P           # 16 key chunks
NPAIR = MJ // 2       # 8 key pair-chunks
INV_SQRT2 = 0.7071067811865476

F32 = mybir.dt.float32
F16 = mybir.dt.float16
F32R = mybir.dt.float32r
BF16 = mybir.dt.bfloat16
F8 = mybir.dt.float8e4
AF = mybir.ActivationFunctionType
DR = mybir.MatmulPerfMode.DoubleRow

EXP_SHIFT = -3.0      # softmax-invariant shift to fit E into fp8e4m3
EXP_SCALE = 1.0 / 16.0  # 1/sqrt(C)

_CACHE = {}


def _build():
    nc = bacc.Bacc("TRN2", target_bir_lowering=False, debug=False, num_devices=8)

    x_d = nc.dram_tensor("x", [C, L], F32, kind="ExternalInput").ap()
    wq_d = nc.dram_tensor("wqT", [C, C], F8, kind="ExternalInput").ap()
    wk_d = nc.dram_tensor("wkT", [C, C], F8, kind="ExternalInput").ap()
    wv_d = nc.dram_tensor("wvT", [C, C], F8, kind="ExternalInput").ap()
    bq_d = nc.dram_tensor("bq", [C], F32, kind="ExternalInput").ap()
    bk_d = nc.dram_tensor("bk", [C], F32, kind="ExternalInput").ap()
    bv_d = nc.dram_tensor("bvt", [C], BF16, kind="ExternalInput").ap()
    y_d = nc.dram_tensor("y", [C, L], F32, kind="ExternalOutput").ap()

    x3 = x_d.rearrange("(co ci) l -> ci co l", ci=P)      # [128, 2, 4096]
    y3 = y_d.rearrange("(co ci) l -> ci co l", ci=P)
    wq3 = wq_d.rearrange("(cc ci) o -> ci cc o", ci=P)    # [128, 2, 256] lhsT
    wk3 = wk_d.rearrange("(cc ci) o -> ci cc o", ci=P)
    wv3 = wv_d.rearrange("(cc ci) o -> ci cc o", ci=P)
    bq2 = bq_d.rearrange("(oc oi) -> oi oc", oi=P)        # [128, 2]
    bk2 = bk_d.rearrange("(oc oi) -> oi oc", oi=P)

    with tile.TileContext(nc) as tc:
        with tc.tile_pool(name="consts", bufs=1) as consts, \
             tc.tile_pool(name="big", bufs=1) as big, \
             tc.tile_pool(name="e", bufs=4) as e_pool, \
             tc.tile_pool(name="tmp", bufs=3) as tmp_pool, \
             tc.tile_pool(name="outp", bufs=2) as out_pool, \
             tc.tile_pool(name="ps", bufs=1, space="PSUM") as ps:

            # ---- constants (host pre-quantized to fp8/bf16) ----
            bq_sb = consts.tile([P, CO], F32)
            bk_sb = consts.tile([P, CO], F32)
            wq_r = consts.tile([P, CO, C], F8)            # DR lhsT/rhs
            wk_r = consts.tile([P, CO, C], F8)
            wv_r = consts.tile([P, CO, C], F8)
            bv_bf = consts.tile([1, C], BF16)

            ones_st = consts.tile([P, CO, 16], F32)
            nc.vector.memset(ones_st, 1.0)
            ones_dr = consts.tile([P, CO, 16], F8)        # DR ones lhsT for Z
            nc.vector.tensor_copy(ones_dr, ones_st)
            ones_row_f = consts.tile([1, P], F32)         # K=1 bcast lhsT (f32r)
            nc.vector.memset(ones_row_f, 1.0)
            ones_row_bf = consts.tile([1, P], BF16)       # bias rank-1 lhsT
            nc.vector.tensor_copy(ones_row_bf, ones_row_f)

            shift_col = consts.tile([P, 1], F32)          # exp bias column
            nc.vector.memset(shift_col, EXP_SHIFT)

            # preload the exp table set during the prologue
            dm = tmp_pool.tile([1, 16], F32, tag="rz", name="dummy_exp")
            nc.scalar.activation(dm, ones_st[0:1, 0, :], AF.Exp,
                                 bias=shift_col[0:1, :])

            # ---- big persistent tensors ----
            x_sb = big.tile([P, NB, CO, LB], F32)         # bank-major
            q_sb = big.tile([P, NB, CO, LB], F8)          # [o, l] bank-major
            xh_sb = big.tile([P, NB, CO, LB // 2], F8)    # [c, m] bank-major
            k_sb = big.tile([P, NB, CO, LB // 2], F8)     # [o, m] bank-major
            vt_sb = big.tile([P, MJ, C], F8)              # [m, o] chunks

            xr_tiles = {}

            def xr_cast(lt):
                xr = e_pool.tile([P, CO, LB], F8, tag="xr", bufs=3,
                                 name=f"xr{lt}")
                nc.vector.tensor_copy(xr, x_bank(lt))
                xr_tiles[lt] = xr

            def q_proj(lt):
                sl = slice(lt * LB, (lt + 1) * LB)
                xr = xr_tiles.pop(lt)
                for oc in range(CO):
                    qp = ps.tile([P, LB], F32, tag="qb", bufs=1,
                                 name=f"qp{lt}_{oc}")
                    nc.tensor.matmul(qp, wq_r[:, :, oc * P:(oc + 1) * P],
                                     xr,
                                     start=True, stop=True, perf_mode=DR)
                    nc.vector.tensor_scalar_add(
                        q_sb[:, lt, oc, :], qp, bq_sb[:, oc:oc + 1])

            # ---- prologue DMAs: tiny fp8 weights land first; x half-bank
            # transfers round-robin over the 3 DMA-capable queues ----
            nc.sync.dma_start(out=wk_r, in_=wk3)
            nc.gpsimd.dma_start(out=wv_r, in_=wv3)
            nc.scalar.dma_start(out=wq_r, in_=wq3)
            nc.scalar.dma_start(out=bq_sb, in_=bq2)
            nc.scalar.dma_start(out=bk_sb, in_=bk2)
            nc.scalar.dma_start(out=bv_bf, in_=bv_d[None, :])
            # x halves: sync takes co0 of banks 0-6, gpsimd co1 of 0-6 plus
            # bank7 co0; scalar only 2 late halves (>6 DMAs on a queue makes
            # dma_start block on semaphore recycling, and scalar must be free
            # early for the vt copies / exps)
            xq = {}
            for j in range(7):
                xq[(j, 0)] = nc.sync
                xq[(j, 1)] = nc.gpsimd
            xq[(7, 0)] = nc.gpsimd
            xq[(6, 1)] = nc.sync
            xq[(6, 0)] = nc.scalar
            xq[(7, 1)] = nc.scalar
            for idx in range(2 * NB):
                j, co = idx // 2, idx % 2
                sl = slice(j * LB, (j + 1) * LB)
                xq[(j, co)].dma_start(out=x_sb[:, j, co, :],
                                      in_=x3[:, co, sl])

            def x_bank(j):
                return x_sb[:, j]

            # ---- per-bank prologue: Haar; K/V proj (fused into lt0) ----
            def bank_prologue(j):
                sl = slice(j * LB, (j + 1) * LB)
                msl = slice(j * (LB // 2), (j + 1) * (LB // 2))  # 256 keys
                # Haar high band (no 1/sqrt2: folded into wk/wv)
                pair = x_bank(j).rearrange(
                    "p c (m two) -> p c m two", two=2)
                nc.vector.tensor_sub(xh_sb[:, j], pair[:, :, :, 0],
                                     pair[:, :, :, 1])

                # K projection: k[o, m] += wk^T xh  (DR over channels)
                kp = ps.tile([P, 2 * LB], F32, tag="score", bufs=2,
                             name=f"kp{j}")
                for oc in range(CO):
                    nc.tensor.matmul(
                        kp[:, oc * 256:(oc + 1) * 256],
                        wk_r[:, :, oc * P:(oc + 1) * P],
                        xh_sb[:, j],
                        start=True, stop=True, perf_mode=DR)
                for oc in range(CO):
                    nc.vector.tensor_scalar_add(
                        k_sb[:, j, oc, :], kp[:, oc * 256:(oc + 1) * 256],
                        bk_sb[:, oc:oc + 1])

                if j == 0:
                    xr_cast(0)
                    q_proj(0)

                # V^T projection: vt[m, o] = xh^T wv + bv (DR over channels)
                vp = ps.tile([P, 512], F32, tag="qb", bufs=1, name=f"vp{j}")
                for t in range(2):
                    mj = 2 * j + t
                    nc.tensor.matmul(
                        vp[:, t * 256:(t + 1) * 256],
                        xh_sb[:, j, :, t * P:(t + 1) * P],
                        wv_r,
                        start=True, stop=False, perf_mode=DR)
                    nc.tensor.matmul(
                        vp[:, t * 256:(t + 1) * 256],
                        ones_row_bf, bv_bf, start=False, stop=True)
                    nc.scalar.copy(vt_sb[:, 2 * j + t, :],
                                   vp[:, t * 256:(t + 1) * 256])

            # ---- attention, one l-tile (512 queries) at a time ----
            LAG = 1
            for lt in range(NB):
                sl = slice(lt * LB, (lt + 1) * LB)
                zp = ps.tile([1, LB], F32, tag="z", bufs=1, name=f"zp{lt}")
                yhp = [ps.tile([P, LB], F32, tag="yh", bufs=2,
                               name=f"yh{lt}_{i}") for i in range(CO)]
                pend = {}

                def consume(t):
                    e2v = pend.pop(t).rearrange("p (two l) -> p two l", two=2)
                    nc.tensor.matmul(zp, ones_dr[:, :, 0:1], e2v,
                                     start=(t == 0), stop=(t == NPAIR - 1),
                                     perf_mode=DR)
                    for oc in range(CO):
                        nc.tensor.matmul(
                            yhp[oc], vt_sb[:, 2 * t:2 * t + 2,
                                           oc * P:(oc + 1) * P], e2v,
                            start=(t == 0), stop=(t == NPAIR - 1),
                            perf_mode=DR)

                for t in range(NPAIR):
                    if lt == 0:
                        # gate lt0's per-bank work at the real (not modeled)
                        # DMA arrival time of bank t so the static schedule
                        # doesn't head-block on late banks
                        arr = 0.008 + 0.0023 * t
                        with tc.tile_wait_until(arr):
                            bank_prologue(t)
                            if t == 1:
                                xr_cast(1)
                    sp = ps.tile([P, 2 * LB], F32, tag="score", bufs=2,
                                 name=f"sp{lt}_{t}")
                    for h in range(2):
                        nc.tensor.matmul(
                            sp[:, h * LB:(h + 1) * LB],
                            k_sb[:, t, :, h * P:(h + 1) * P],
                            q_sb[:, lt],
                            start=True, stop=True, perf_mode=DR)
                    if t == 5 and lt + 2 < NB:
                        xr_cast(lt + 2)
                    e2 = e_pool.tile([P, 2 * LB], F8, tag="e",
                                     name=f"e{lt}_{t}")
                    nc.scalar.activation(e2, sp, AF.Exp,
                                         bias=shift_col, scale=EXP_SCALE)
                    pend[t] = e2
                    if t == 2 and lt + 1 < NB:
                        q_proj(lt + 1)
                    if t >= LAG:
                        consume(t - LAG)
                for t in range(NPAIR - LAG, NPAIR):
                    consume(t)

                # normalize + residual
                bpp = ps.tile([P, LB], F32, tag="qb", bufs=1, name=f"bp{lt}")
                rz = tmp_pool.tile([1, LB], F32, tag="rz", name=f"rz{lt}")
                nc.vector.reciprocal_approx_fast(out=rz, in_=zp)
                rz_bf = tmp_pool.tile([1, LB], BF16, tag="rzb",
                                      name=f"rzb{lt}")
                nc.vector.tensor_copy(rz_bf, rz)
                nc.tensor.matmul(bpp, ones_row_bf, rz_bf,
                                 start=True, stop=True)
                b_sb = tmp_pool.tile([P, LB], F32, tag="bsb", name=f"bsb{lt}")
                nc.vector.tensor_copy(b_sb, bpp)
                o_sb = out_pool.tile([P, CO, LB], F32, tag="o",
                                     name=f"o{lt}")
                for oc in range(CO):
                    t_sb = tmp_pool.tile([P, LB], F32, tag="t",
                                         name=f"t{lt}_{oc}")
                    nc.vector.tensor_mul(t_sb, yhp[oc], b_sb)
                    nc.vector.tensor_add(o_sb[:, oc, :], t_sb,
                                         x_bank(lt)[:, oc, :])
                    (nc.sync, nc.gpsimd)[(lt + oc) % 2].dma_start(
                        out=y3[:, oc, sl], in_=o_sb[:, oc, :])

    nc.compile()
    return nc


def _get_nc():
    if "nc" not in _CACHE:
        _CACHE["nc"] = _build()
    return _CACHE["nc"]


def _prep_inputs(x, Wq, bq, Wk, bk, Wv, bv, attn_gate):
    x = np.asarray(x, dtype=np.float32)
    Wq = np.asarray(Wq, dtype=np.float32)
    Wk = np.asarray(Wk, dtype=np.float32)
    Wv = np.asarray(Wv, dtype=np.float32)
    bq = np.asarray(bq, dtype=np.float32)
    bk = np.asarray(bk, dtype=np.float32)
    bv = np.asarray(bv, dtype=np.float32)
    gate = float(np.tanh(np.asarray(attn_gate, dtype=np.float64))[0])

    import ml_dtypes
    f8np = ml_dtypes.float8_e4m3
    # lhsT layouts [c_in, c_out]; fold haar 1/sqrt(2) into k and v,
    # tanh(gate) into v.  1/sqrt(C) rides the exp free-affine on device.
    # Weights pre-quantized on host to the device matmul dtypes.
    wqT = np.ascontiguousarray(Wq.T).astype(f8np)
    wkT = np.ascontiguousarray(Wk.T * np.float32(INV_SQRT2)).astype(f8np)
    wvT = np.ascontiguousarray(Wv.T * np.float32(INV_SQRT2 * gate)).astype(
        f8np)
    bv_t = (bv * np.float32(gate)).astype(ml_dtypes.bfloat16)
    return [{
        "x": np.ascontiguousarray(x[b]),
        "wqT": wqT, "wkT": wkT, "wvT": wvT,
        "bq": bq, "bk": bk, "bvt": bv_t,
    } for b in range(B)]


def kernel(x, Wq, bq, Wk, bk, Wv, bv, attn_gate, _run_kwargs=None):
    in_maps = _prep_inputs(x, Wq, bq, Wk, bk, Wv, bv, attn_gate)
    nc = _get_nc()
    res = bass_utils.run_bass_kernel_spmd(
        nc, in_maps, core_ids=list(range(B)), **(_run_kwargs or {}))
    out = np.stack([res.results[b]["y"] for b in range(B)]).astype(np.float32)
    if _run_kwargs:
        kernel.last_results = res
    return out


# revision 32
# speedup vs baseline: 1.0754x; 1.0073x over previous
"""Trainium2 Bass kernel for nn_HFGA_54606214201918.

Computation (per batch element b, C=256 channels, L=4096 positions):
    xh  = (x[:, 0::2] - x[:, 1::2]) / sqrt(2)          # Haar high band  [C, L/2]
    q   = Wq @ x + bq                                  # [C, L]
    k   = Wk @ xh + bk                                 # [C, L/2]
    v   = Wv @ xh + bv                                 # [C, L/2]
    attn = softmax_over_keys((k^T q) / sqrt(C))        # [L/2, L]
    out = (v @ attn) * tanh(gate) + x

Sharding: data-parallel over batch B=8 across the 8 NeuronCores (one batch
element per core); weights are broadcast. No collectives needed.

Per-core algorithm:
  - The two big matmuls (scores k^T q and yh = v @ E) plus the K/V projections
    run in fp8e4m3 with DoubleRow perf mode (K=256 contraction per MM, 2
    MACs/cell/cycle). The Q projection runs in float32r directly from the
    fp32 x tile (saves a DVE quantize pass over x).
  - scores land in [keys m, queries l] layout, two 128-key chunks per 2-bank
    PSUM tile, so one ACTIVATE(Exp) drains 1024 f32/partition per instruction
    and emits the fp8 E pair tile in exactly the DoubleRow rhs layout.
  - 1/sqrt(C) and the softmax shift (-3, range-fitting exp into fp8) ride the
    ACT free affine: E = exp(s/16 - 3); the shift cancels in softmax.
  - softmax denominator Z[l] via a DoubleRow ones-column matmul accumulated
    over key pairs; normalization applied to the small output (v@E) with a
    K=1 f32r broadcast matmul, fused with the residual add on the DVE.
  - Prologue pipelines per 512-column x bank: DMA -> Haar sub -> K/V
    projections, so the PE starts ~2.5us in. Q projection for l-tile lt+1 is
    emitted inside l-tile lt's attention loop.
  - 1/sqrt(2) and tanh(gate) are folded into Wk/Wv/bv on host.
"""
import sys

if '/opt/trn_rl_repo' not in sys.path:
    sys.path.insert(0, '/opt/trn_rl_repo')

import numpy as np

import concourse.bass as bass
import concourse.tile as tile
from concourse import bacc, mybir
from concourse import bass_utils

B, C, L = 8, 256, 4096
M = L // 2            # 2048 keys
P = 128               # partitions
CO = C // P           # 2 channel chunks
LB = 512              # l-tile (one PSUM bank of fp32)
NB = L // LB          # 8 l-tiles
MJ = M // P           # 16 key chunks
NPAIR = MJ // 2       # 8 key pair-chunks
INV_SQRT2 = 0.7071067811865476

F32 = mybir.dt.float32
F16 = mybir.dt.float16
F32R = mybir.dt.float32r
BF16 = mybir.dt.bfloat16
F8 = mybir.dt.float8e4
AF = mybir.ActivationFunctionType
DR = mybir.MatmulPerfMode.DoubleRow

EXP_SHIFT = -3.0      # softmax-invariant shift to fit E into fp8e4m3
EXP_SCALE = 1.0 / 16.0  # 1/sqrt(C)

_CACHE = {}


def _build():
    nc = bacc.Bacc("TRN2", target_bir_lowering=False, debug=False, num_devices=8)

    x_d = nc.dram_tensor("x", [C, L], F32, kind="ExternalInput").ap()
    wq_d = nc.dram_tensor("wqT", [C, C], F8, kind="ExternalInput").ap()
    wk_d = nc.dram_tensor("wkT", [C, C], F8, kind="ExternalInput").ap()
    wv_d = nc.dram_tensor("wvT", [C, C], F8, kind="ExternalInput").ap()
    bq_d = nc.dram_tensor("bq", [C], F32, kind="ExternalInput").ap()
    bk_d = nc.dram_tensor("bk", [C], F32, kind="ExternalInput").ap()
    bv_d = nc.dram_tensor("bvt", [C], BF16, kind="ExternalInput").ap()
    y_d = nc.dram_tensor("y", [C, L], F32, kind="ExternalOutput").ap()

    x3 = x_d.rearrange("(co ci) l -> ci co l", ci=P)      # [128, 2, 4096]
    y3 = y_d.rearrange("(co ci) l -> ci co l", ci=P)
    wq3 = wq_d.rearrange("(cc ci) o -> ci cc o", ci=P)    # [128, 2, 256] lhsT
    wk3 = wk_d.rearrange("(cc ci) o -> ci cc o", ci=P)
    wv3 = wv_d.rearrange("(cc ci) o -> ci cc o", ci=P)
    bq2 = bq_d.rearrange("(oc oi) -> oi oc", oi=P)        # [128, 2]
    bk2 = bk_d.rearrange("(oc oi) -> oi oc", oi=P)

    with tile.TileContext(nc) as tc:
        with tc.tile_pool(name="consts", bufs=1) as consts, \
             tc.tile_pool(name="big", bufs=1) as big, \
             tc.tile_pool(name="e", bufs=4) as e_pool, \
             tc.tile_pool(name="tmp", bufs=3) as tmp_pool, \
             tc.tile_pool(name="outp", bufs=2) as out_pool, \
             tc.tile_pool(name="ps", bufs=1, space="PSUM") as ps:

            # ---- constants (host pre-quantized to fp8/bf16) ----
            bq_sb = consts.tile([P, CO], F32)
            bk_sb = consts.tile([P, CO], F32)
            wq_r = consts.tile([P, CO, C], F8)            # DR lhsT/rhs
            wk_r = consts.tile([P, CO, C], F8)
            wv_r = consts.tile([P, CO, C], F8)
            bv_bf = consts.tile([1, C], BF16)

            ones_st = consts.tile([P, CO, 16], F32)
            nc.vector.memset(ones_st, 1.0)
            ones_dr = consts.tile([P, CO, 16], F8)        # DR ones lhsT for Z
            nc.vector.tensor_copy(ones_dr, ones_st)
            ones_row_f = consts.tile([1, P], F32)         # K=1 bcast lhsT (f32r)
            nc.vector.memset(ones_row_f, 1.0)
            ones_row_bf = consts.tile([1, P], BF16)       # bias rank-1 lhsT
            nc.vector.tensor_copy(ones_row_bf, ones_row_f)

            shift_col = consts.tile([P, 1], F32)          # exp bias column
            nc.vector.memset(shift_col, EXP_SHIFT)

            # preload the exp table set during the prologue
            dm = tmp_pool.tile([1, 16], F32, tag="rz", name="dummy_exp")
            nc.scalar.activation(dm, ones_st[0:1, 0, :], AF.Exp,
                                 bias=shift_col[0:1, :])

            # ---- big persistent tensors ----
            x_sb = big.tile([P, NB, CO, LB], F32)         # bank-major
            q_sb = big.tile([P, NB, CO, LB], F8)          # [o, l] bank-major
            xh_sb = big.tile([P, NB, CO, LB // 2], F8)    # [c, m] bank-major
            k_sb = big.tile([P, NB, CO, LB // 2], F8)     # [o, m] bank-major
            vt_sb = big.tile([P, MJ, C], F8)              # [m, o] chunks

            xr_tiles = {}

            def xr_cast(lt):
                xr = e_pool.tile([P, CO, LB], F8, tag="xr", bufs=3,
                                 name=f"xr{lt}")
                nc.vector.tensor_copy(xr, x_bank(lt))
                xr_tiles[lt] = xr

            def q_proj(lt):
                sl = slice(lt * LB, (lt + 1) * LB)
                xr = xr_tiles.pop(lt)
                for oc in range(CO):
                    qp = ps.tile([P, LB], F32, tag="qb", bufs=1,
                                 name=f"qp{lt}_{oc}")
                    nc.tensor.matmul(qp, wq_r[:, :, oc * P:(oc + 1) * P],
                                     xr,
                                     start=True, stop=True, perf_mode=DR)
                    nc.vector.tensor_scalar_add(
                        q_sb[:, lt, oc, :], qp, bq_sb[:, oc:oc + 1])

            # ---- prologue DMAs: tiny fp8 weights land first; x half-bank
            # transfers round-robin over the 3 DMA-capable queues ----
            nc.sync.dma_start(out=wk_r, in_=wk3)
            nc.gpsimd.dma_start(out=wv_r, in_=wv3)
            nc.scalar.dma_start(out=wq_r, in_=wq3)
            nc.scalar.dma_start(out=bq_sb, in_=bq2)
            nc.scalar.dma_start(out=bk_sb, in_=bk2)
            nc.scalar.dma_start(out=bv_bf, in_=bv_d[None, :])
            # x halves: sync takes co0 of banks 0-6, gpsimd co1 of 0-6 plus
            # bank7 co0; scalar only 2 late halves (>6 DMAs on a queue makes
            # dma_start block on semaphore recycling, and scalar must be free
            # early for the vt copies / exps)
            xq = {}
            for j in range(7):
                xq[(j, 0)] = nc.sync
                xq[(j, 1)] = nc.gpsimd
            xq[(7, 0)] = nc.gpsimd
            xq[(6, 1)] = nc.sync
            xq[(6, 0)] = nc.scalar
            xq[(7, 1)] = nc.scalar
            for idx in range(2 * NB):
                j, co = idx // 2, idx % 2
                sl = slice(j * LB, (j + 1) * LB)
                xq[(j, co)].dma_start(out=x_sb[:, j, co, :],
                                      in_=x3[:, co, sl])

            def x_bank(j):
                return x_sb[:, j]

            # ---- per-bank prologue: Haar; K/V proj (fused into lt0) ----
            def bank_prologue(j):
                sl = slice(j * LB, (j + 1) * LB)
                msl = slice(j * (LB // 2), (j + 1) * (LB // 2))  # 256 keys
                # Haar high band (no 1/sqrt2: folded into wk/wv)
                pair = x_bank(j).rearrange(
                    "p c (m two) -> p c m two", two=2)
                nc.vector.tensor_sub(xh_sb[:, j], pair[:, :, :, 0],
                                     pair[:, :, :, 1])

                # K projection: k[o, m] += wk^T xh  (DR over channels)
                kp = ps.tile([P, 2 * LB], F32, tag="score", bufs=2,
                             name=f"kp{j}")
                for oc in range(CO):
                    nc.tensor.matmul(
                        kp[:, oc * 256:(oc + 1) * 256],
                        wk_r[:, :, oc * P:(oc + 1) * P],
                        xh_sb[:, j],
                        start=True, stop=True, perf_mode=DR)
                for oc in range(CO):
                    nc.vector.tensor_scalar_add(
                        k_sb[:, j, oc, :], kp[:, oc * 256:(oc + 1) * 256],
                        bk_sb[:, oc:oc + 1])

                if j == 0:
                    xr_cast(0)
                    q_proj(0)

                # V^T projection: vt[m, o] = xh^T wv + bv (DR over channels)
                vp = ps.tile([P, 512], F32, tag="qb", bufs=1, name=f"vp{j}")
                for t in range(2):
                    mj = 2 * j + t
                    nc.tensor.matmul(
                        vp[:, t * 256:(t + 1) * 256],
                        xh_sb[:, j, :, t * P:(t + 1) * P],
                        wv_r,
                        start=True, stop=False, perf_mode=DR)
                    nc.tensor.matmul(
                        vp[:, t * 256:(t + 1) * 256],
                        ones_row_bf, bv_bf, start=False, stop=True)
                    nc.scalar.copy(vt_sb[:, 2 * j + t, :],
                                   vp[:, t * 256:(t + 1) * 256])

            # ---- attention, one l-tile (512 queries) at a time ----
            LAG = 1
            for lt in range(NB):
                sl = slice(lt * LB, (lt + 1) * LB)
                zp = ps.tile([1, LB], F32, tag="z", bufs=1, name=f"zp{lt}")
                yhp = [ps.tile([P, LB], F32, tag="yh", bufs=2,
                               name=f"yh{lt}_{i}") for i in range(CO)]
                pend = {}

                def consume(t):
                    e2v = pend.pop(t).rearrange("p (two l) -> p two l", two=2)
                    nc.tensor.matmul(zp, ones_dr[:, :, 0:1], e2v,
                                     start=(t == 0), stop=(t == NPAIR - 1),
                                     perf_mode=DR)
                    for oc in range(CO):
                        nc.tensor.matmul(
                            yhp[oc], vt_sb[:, 2 * t:2 * t + 2,
                                           oc * P:(oc + 1) * P], e2v,
                            start=(t == 0), stop=(t == NPAIR - 1),
                            perf_mode=DR)

                for t in range(NPAIR):
                    if lt == 0:
                        # gate lt0's per-bank work at the real (not modeled)
                        # DMA arrival time of bank t so the static schedule
                        # doesn't head-block on late banks
                        arr = 0.008 + 0.0023 * t
                        with tc.tile_wait_until(arr):
                            bank_prologue(t)
                            if t == 1:
                                xr_cast(1)
                    sp = ps.tile([P, 2 * LB], F32, tag="score", bufs=2,
                                 name=f"sp{lt}_{t}")
                    for h in range(2):
                        nc.tensor.matmul(
                            sp[:, h * LB:(h + 1) * LB],
                            k_sb[:, t, :, h * P:(h + 1) * P],
                            q_sb[:, lt],
                            start=True, stop=True, perf_mode=DR)
                    if t == 3 and lt + 2 < NB:
                        xr_cast(lt + 2)
                    e2 = e_pool.tile([P, 2 * LB], F8, tag="e",
                                     name=f"e{lt}_{t}")
                    nc.scalar.activation(e2, sp, AF.Exp,
                                         bias=shift_col, scale=EXP_SCALE)
                    pend[t] = e2
                    if t == 1 and lt + 1 < NB:
                        q_proj(lt + 1)
                    if t >= LAG:
                        consume(t - LAG)
                for t in range(NPAIR - LAG, NPAIR):
                    consume(t)

                # normalize + residual
                bpp = ps.tile([P, LB], F32, tag="qb", bufs=1, name=f"bp{lt}")
                rz = tmp_pool.tile([1, LB], F32, tag="rz", name=f"rz{lt}")
                nc.vector.reciprocal_approx_fast(out=rz, in_=zp)
                rz_bf = tmp_pool.tile([1, LB], BF16, tag="rzb",
                                      name=f"rzb{lt}")
                nc.vector.tensor_copy(rz_bf, rz)
                nc.tensor.matmul(bpp, ones_row_bf, rz_bf,
                                 start=True, stop=True)
                b_sb = tmp_pool.tile([P, LB], F32, tag="bsb", name=f"bsb{lt}")
                nc.vector.tensor_copy(b_sb, bpp)
                o_sb = out_pool.tile([P, CO, LB], F32, tag="o",
                                     name=f"o{lt}")
                for oc in range(CO):
                    t_sb = tmp_pool.tile([P, LB], F32, tag="t",
                                         name=f"t{lt}_{oc}")
                    nc.vector.tensor_mul(t_sb, yhp[oc], b_sb)
                    nc.vector.tensor_add(o_sb[:, oc, :], t_sb,
                                         x_bank(lt)[:, oc, :])
                    (nc.sync, nc.gpsimd)[(lt + oc) % 2].dma_start(
                        out=y3[:, oc, sl], in_=o_sb[:, oc, :])

    nc.compile()
    return nc


def _get_nc():
    if "nc" not in _CACHE:
        _CACHE["nc"] = _build()
    return _CACHE["nc"]


def _prep_inputs(x, Wq, bq, Wk, bk, Wv, bv, attn_gate):
    x = np.asarray(x, dtype=np.float32)
    Wq = np.asarray(Wq, dtype=np.float32)
    Wk = np.asarray(Wk, dtype=np.float32)
    Wv = np.asarray(Wv, dtype=np.float32)
    bq = np.asarray(bq, dtype=np.float32)
    bk = np.asarray(bk, dtype=np.float32)
    bv = np.asarray(bv, dtype=np.float32)
    gate = float(np.tanh(np.asarray(attn_gate, dtype=np.float64))[0])

    import ml_dtypes
    f8np = ml_dtypes.float8_e4m3
    # lhsT layouts [c_in, c_out]; fold haar 1/sqrt(2) into k and v,
    # tanh(gate) into v.  1/sqrt(C) rides the exp free-affine on device.
    # Weights pre-quantized on host to the device matmul dtypes.
    wqT = np.ascontiguousarray(Wq.T).astype(f8np)
    wkT = np.ascontiguousarray(Wk.T * np.float32(INV_SQRT2)).astype(f8np)
    wvT = np.ascontiguousarray(Wv.T * np.float32(INV_SQRT2 * gate)).astype(
        f8np)
    bv_t = (bv * np.float32(gate)).astype(ml_dtypes.bfloat16)
    return [{
        "x": np.ascontiguousarray(x[b]),
        "wqT": wqT, "wkT": wkT, "wvT": wvT,
        "bq": bq, "bk": bk, "bvt": bv_t,
    } for b in range(B)]


def kernel(x, Wq, bq, Wk, bk, Wv, bv, attn_gate, _run_kwargs=None):
    in_maps = _prep_inputs(x, Wq, bq, Wk, bk, Wv, bv, attn_gate)
    nc = _get_nc()
    res = bass_utils.run_bass_kernel_spmd(
        nc, in_maps, core_ids=list(range(B)), **(_run_kwargs or {}))
    out = np.stack([res.results[b]["y"] for b in range(B)]).astype(np.float32)
    if _run_kwargs:
        kernel.last_results = res
    return out
